# revision 4
# baseline (speedup 1.0000x reference)
"""Causal self-attention (B=2, T=2048, C=1024, H=16, D=64) on 8 trn2 cores.

Sharding: core c -> batch b = c // 4, head-group g = c % 4 (4 heads each).
Data-parallel over B, tensor-parallel (Megatron) over heads for the
qkv / proj linears. Each core computes its head-group's attention and a
partial output projection; the host sums the 4 partials per batch and
adds the proj bias.

Everything on-device is done in transposed [feature, token] space:
  qk^T = Wqk @ x^T                       (PE; bias added by ACT on evacuate)
  v    = x @ Wv^T (+ ones column)        (PE; K=1 matmul adds bias + ones)
  att^T[k, q] = k^T(head)^T . q(head)    (PE, K=64; causal tiles skipped)
  P = exp(att^T + additive causal mask)  (ACT; no max-subtraction needed,
                                          |logits| <~ 10 so fp32 exp is safe)
  rawout^T[d+1, q] = v_aug^T @ P         (PE accumulate over k chunks; the
                                          ones column makes row 64 = sum_k P
                                          = softmax denominator, for free)
  out^T = rawout^T[:64] * (1/denom)      (DVE recip + PE K=1 broadcast + DVE)
  y^T(partial) = Wp_g^T.T @ out^T        (PE)
"""

import os
import sys
import types

for _p in ("/opt/trn_rl_repo", "/root/.axon_site", "/root/.axon_site/_ro/trn_rl_repo"):
    if os.path.isdir(_p) and _p not in sys.path:
        sys.path.append(_p)

import numpy as np

import concourse.bacc as bacc
import concourse.mybir as mybir
import concourse.tile as tile
from concourse.bass_utils import run_bass_kernel_spmd

# ── problem constants (hardcoded; spec.json not available at grade time) ──
B, T, C = 2, 2048, 1024
H, D = 16, 64
N_CORES = 8
HPG = 4                 # heads per group (per core)
CG = HPG * D            # 256 channels per head-group
NT = T // 512           # 4 token chunks of 512
KC = C // 128           # 8 contraction tiles for C
NEG = -1.0e4            # additive causal mask value (exp -> exact 0.0)

F32 = mybir.dt.float32
_trace_flag = [False]   # test.py can flip this to capture a profile
_last_results = [None]


def _ensure_ntff_hook():
    """Install the NTFF profile hook shim (container's antenv lacks it)."""
    if "antenv.axon_hooks" in sys.modules:
        return
    try:
        from trn_agent_boot.trn_boot import _ntff_profile_via_ctypes
    except Exception:
        return
    mod = types.ModuleType("antenv.axon_hooks")
    hook = [None]
    mod.set_axon_ntff_profile_hook = lambda h: hook.__setitem__(0, h)
    mod.get_axon_ntff_profile_hook = lambda: hook[0]
    sys.modules["antenv.axon_hooks"] = mod
    so = "/opt/axon/libaxon_pjrt.so"
    if os.path.exists(so):
        mod.set_axon_ntff_profile_hook(_ntff_profile_via_ctypes(so))


def build_nc():
    nc = bacc.Bacc("TRN2", target_bir_lowering=False, debug=False,
                   num_devices=N_CORES)

    xt_d = nc.dram_tensor("xt", [C, T], F32, kind="ExternalInput").ap()
    wqk_d = nc.dram_tensor("wqk", [C, 2 * CG], F32, kind="ExternalInput").ap()
    bqk_d = nc.dram_tensor("bqk", [2 * CG, 1], F32, kind="ExternalInput").ap()
    wv_d = nc.dram_tensor("wv", [C, HPG * 65], F32, kind="ExternalInput").ap()
    bv_d = nc.dram_tensor("bv", [1, HPG * 65], F32, kind="ExternalInput").ap()
    wp_d = nc.dram_tensor("wp", [CG, C], F32, kind="ExternalInput").ap()
    mask_d = nc.dram_tensor("mask", [4 * 128, 512], F32, kind="ExternalInput").ap()
    yt_d = nc.dram_tensor("yt", [C, T], F32, kind="ExternalOutput").ap()

    with tile.TileContext(nc) as tc:
        with tc.tile_pool(name="const", bufs=1) as cp:
            # ── persistent SBUF residents ──
            xt = [cp.tile([128, T], F32, tag=f"xt{k}", name=f"xt{k}") for k in range(KC)]
            wqk = [cp.tile([128, 2 * CG], F32, tag=f"wqk{k}", name=f"wqk{k}") for k in range(KC)]
            wv = [cp.tile([128, HPG * 65], F32, tag=f"wv{k}", name=f"wv{k}") for k in range(KC)]
            bqk = [cp.tile([128, 1], F32, tag=f"bqk{m}", name=f"bqk{m}") for m in range(4)]
            bv = cp.tile([1, HPG * 65], F32, tag="bv")
            wp = [cp.tile([128, C], F32, tag=f"wp{k}", name=f"wp{k}") for k in range(2)]
            masks = [cp.tile([128, 512], F32, tag=f"mask{p}", name=f"mask{p}") for p in range(4)]
            ones = cp.tile([1, 128], F32, tag="ones")
            qk = [cp.tile([128, T], F32, tag=f"qk{m}", name=f"qk{m}") for m in range(4)]
            v_sb = [cp.tile([128, HPG * 65], F32, tag=f"v{m}", name=f"v{m}") for m in range(T // 128)]
            outT = [cp.tile([128, T], F32, tag=f"outT{k}", name=f"outT{k}") for k in range(2)]

            for k in range(KC):
                nc.sync.dma_start(xt[k][:], xt_d[128 * k:128 * (k + 1), :])
                nc.sync.dma_start(wqk[k][:], wqk_d[128 * k:128 * (k + 1), :])
                nc.sync.dma_start(wv[k][:], wv_d[128 * k:128 * (k + 1), :])
            for m in range(4):
                nc.sync.dma_start(bqk[m][:], bqk_d[128 * m:128 * (m + 1), :])
            nc.sync.dma_start(bv[:], bv_d[:])
            for k in range(2):
                nc.sync.dma_start(wp[k][:], wp_d[128 * k:128 * (k + 1), :])
            for p in range(4):
                nc.sync.dma_start(masks[p][:], mask_d[128 * p:128 * (p + 1), :])
            nc.vector.memset(ones[:], 1.0)

            # ── stage B: qk^T [512, T] = wqk.T @ xt, bias on evacuation ──
            with tc.tile_pool(name="psB", bufs=3, space="PSUM") as psB:
                for mf in range(4):
                    for nt in range(NT):
                        ps = psB.tile([128, 512], F32)
                        for k in range(KC):
                            nc.tensor.matmul(
                                ps[:], wqk[k][:, 128 * mf:128 * (mf + 1)],
                                xt[k][:, 512 * nt:512 * (nt + 1)],
                                start=(k == 0), stop=(k == KC - 1))
                        nc.scalar.activation(
                            qk[mf][:, 512 * nt:512 * (nt + 1)], ps[:],
                            mybir.ActivationFunctionType.Identity, bias=bqk[mf][:])

                # ── stage C: v_aug [T, 260] = xt.T @ wv (+ ones col via K=1) ──
                for mt in range(T // 128):
                    ps = psB.tile([128, HPG * 65], F32, tag="psv")
                    for k in range(KC):
                        nc.tensor.matmul(
                            ps[:], xt[k][:, 128 * mt:128 * (mt + 1)], wv[k][:],
                            start=(k == 0), stop=False)
                    nc.tensor.matmul(ps[:], ones[:, :], bv[:], start=False,
                                     stop=True)
                    nc.vector.tensor_copy(v_sb[mt][:], ps[:])

            # ── stage D: attention per (head, q-chunk) ──
            with (
                tc.tile_pool(name="psA", bufs=3, space="PSUM") as psA,
                tc.tile_pool(name="psAV", bufs=2, space="PSUM") as psAV,
                tc.tile_pool(name="psBC", bufs=2, space="PSUM") as psBC,
                tc.tile_pool(name="expp", bufs=3) as expp,
                tc.tile_pool(name="recp", bufs=2) as recp,
            ):
                for h in range(HPG):
                    qtile, off = h // 2, 64 * (h % 2)
                    ktile = 2 + qtile
                    for j in range(NT):
                        n_i = 4 * (j + 1)   # causal: k-chunks 0 .. 4j+3
                        avps = psAV.tile([65, 512], F32)
                        pending = None      # software pipeline: av lags attT by 1
                        for i in range(n_i):
                            aps = psA.tile([128, 512], F32)
                            nc.tensor.matmul(
                                aps[:],
                                qk[ktile][off:off + 64, 128 * i:128 * (i + 1)],
                                qk[qtile][off:off + 64, 512 * j:512 * (j + 1)],
                                start=True, stop=True)
                            if i >= 4 * j:  # diagonal tile: additive causal mask
                                nc.vector.tensor_add(aps[:], aps[:],
                                                     masks[i - 4 * j][:])
                            et = expp.tile([128, 512], F32)
                            nc.scalar.activation(et[:], aps[:],
                                                 mybir.ActivationFunctionType.Exp)
                            if pending is not None:
                                pi, pet = pending
                                nc.tensor.matmul(
                                    avps[:], v_sb[pi][:, 65 * h:65 * h + 65],
                                    pet[:], start=(pi == 0), stop=False)
                            pending = (i, et)
                        pi, pet = pending
                        nc.tensor.matmul(
                            avps[:], v_sb[pi][:, 65 * h:65 * h + 65], pet[:],
                            start=(pi == 0), stop=True)

                        rec = recp.tile([1, 512], F32)
                        nc.vector.reciprocal(rec[:], avps[64:65, :])
                        bc = psBC.tile([64, 512], F32)
                        nc.tensor.matmul(bc[:], ones[:, 0:64], rec[:],
                                         start=True, stop=True)
                        bc_sb = recp.tile([64, 512], F32, tag="bc_sb")
                        nc.vector.tensor_copy(bc_sb[:], bc[:])
                        nc.vector.tensor_mul(
                            outT[qtile][off:off + 64, 512 * j:512 * (j + 1)],
                            avps[0:64, :], bc_sb[:])

            # ── stage E: y^T partial [C, T] = wp.T @ outT ──
            with (
                tc.tile_pool(name="psP", bufs=3, space="PSUM") as psP,
                tc.tile_pool(name="outp", bufs=3) as outp,
            ):
                for mo in range(8):
                    for nt in range(NT):
                        ps = psP.tile([128, 512], F32)
                        for k in range(2):
                            nc.tensor.matmul(
                                ps[:], wp[k][:, 128 * mo:128 * (mo + 1)],
                                outT[k][:, 512 * nt:512 * (nt + 1)],
                                start=(k == 0), stop=(k == 1))
                        ot = outp.tile([128, 512], F32)
                        nc.scalar.activation(ot[:], ps[:],
                                             mybir.ActivationFunctionType.Copy)
                        nc.sync.dma_start(
                            yt_d[128 * mo:128 * (mo + 1),
                                 512 * nt:512 * (nt + 1)], ot[:])

    nc.compile()
    return nc


def _shard_inputs(x, w_qkv, b_qkv, w_proj):
    scale = 1.0 / np.sqrt(D)   # 0.125, exact power of two
    in_maps = []
    mask = np.empty((4 * 128, 512), np.float32)
    for p in range(4):
        r = np.arange(128)[:, None]
        c = np.arange(512)[None, :]
        mask[128 * p:128 * (p + 1)] = np.where(c >= r + 128 * p, 0.0, NEG)
    for core in range(N_CORES):
        b, g = divmod(core, HPG)
        qs = slice(CG * g, CG * (g + 1))
        ks = slice(C + CG * g, C + CG * (g + 1))
        vs = slice(2 * C + CG * g, 2 * C + CG * (g + 1))
        wqk = np.concatenate([w_qkv[qs] * scale, w_qkv[ks]], axis=0).T
        bqk = np.concatenate([b_qkv[qs] * scale, b_qkv[ks]])[:, None]
        wv_base = w_qkv[vs].T          # [C, 256]
        wv = np.zeros((C, HPG * 65), np.float32)
        bv = np.zeros((1, HPG * 65), np.float32)
        for h in range(HPG):
            wv[:, 65 * h:65 * h + 64] = wv_base[:, 64 * h:64 * h + 64]
            bv[0, 65 * h:65 * h + 64] = b_qkv[vs][64 * h:64 * h + 64]
            bv[0, 65 * h + 64] = 1.0
        in_maps.append({
            "xt": np.ascontiguousarray(x[b].T, np.float32),
            "wqk": np.ascontiguousarray(wqk, np.float32),
            "bqk": np.ascontiguousarray(bqk, np.float32),
            "wv": wv,
            "bv": bv,
            "wp": np.ascontiguousarray(w_proj[:, CG * g:CG * (g + 1)].T,
                                       np.float32),
            "mask": mask,
        })
    return in_maps


def kernel(x, w_qkv, b_qkv, w_proj, b_proj):
    x = np.asarray(x, np.float32)
    w_qkv = np.asarray(w_qkv, np.float32)
    b_qkv = np.asarray(b_qkv, np.float32)
    w_proj = np.asarray(w_proj, np.float32)
    b_proj = np.asarray(b_proj, np.float32)

    nc = build_nc()
    in_maps = _shard_inputs(x, w_qkv, b_qkv, w_proj)
    if _trace_flag[0]:
        _ensure_ntff_hook()
    res = run_bass_kernel_spmd(nc, in_maps, core_ids=list(range(N_CORES)),
                               trace=_trace_flag[0])
    _last_results[0] = res

    y = np.empty((B, T, C), np.float32)
    for b in range(B):
        acc = np.zeros((C, T), np.float32)
        for g in range(HPG):
            acc += res.results[HPG * b + g]["yt"]
        y[b] = acc.T + b_proj[None, :]
    return y


# revision 11
# speedup vs baseline: 2.4217x; 2.4217x over previous
"""Causal self-attention (B=2, T=2048, C=1024, H=16, D=64) on 8 trn2 cores.

Sharding: core c -> batch b = c // 4, head-group g = c % 4 (4 heads each).
Data-parallel over B, tensor-parallel (Megatron) over heads for the
qkv / proj linears. Each core computes its head-group's attention and a
partial output projection; the host sums the 4 partials per batch and
adds the proj bias.

Everything on-device is done in transposed [feature, token] space:
  qk^T = Wqk @ x^T                       (PE; bias added by ACT on evacuate)
  v    = x @ Wv^T (+ ones column)        (PE; K=1 matmul adds bias + ones)
  att^T[k, q] = k^T(head)^T . q(head)    (PE, K=64; causal tiles skipped)
  P = exp(att^T + additive causal mask)  (ACT; no max-subtraction needed,
                                          |logits| <~ 10 so fp32 exp is safe)
  rawout^T[d+1, q] = v_aug^T @ P         (PE accumulate over k chunks; the
                                          ones column makes row 64 = sum_k P
                                          = softmax denominator, for free)
  out^T = rawout^T[:64] * (1/denom)      (DVE recip + PE K=1 broadcast + DVE)
  y^T(partial) = Wp_g^T.T @ out^T        (PE)
"""

import os
import sys
import types

for _p in ("/opt/trn_rl_repo", "/root/.axon_site", "/root/.axon_site/_ro/trn_rl_repo"):
    if os.path.isdir(_p) and _p not in sys.path:
        sys.path.append(_p)

import numpy as np

import concourse.bacc as bacc
import concourse.bass as bass
import concourse.mybir as mybir
import concourse.tile as tile
from concourse.bass_utils import run_bass_kernel_spmd

# ── problem constants (hardcoded; spec.json not available at grade time) ──
B, T, C = 2, 2048, 1024
H, D = 16, 64
N_CORES = 8
HPG = 4                 # heads per group (per core)
CG = HPG * D            # 256 channels per head-group
NT = T // 512           # 4 token chunks of 512
KC = C // 128           # 8 contraction tiles for C
NEG = -1.0e4            # (unused) additive mask value

F32 = mybir.dt.float32
F32R = mybir.dt.float32r
# per-stage matmul operand dtype: float32 (exact, 4 cyc/row) or float32r
# (tf32-like, 1 cyc/row at N>=256). Overridable for A/B testing.
MMDT = {
    "qk": F32R, "v": F32R, "att": F32R, "av": F32R, "proj": F32R, "k1": F32,
}
_trace_flag = [False]   # test.py can flip this to capture a profile
_last_results = [None]


def _mm(nc, out, lhsT, rhs, stage, **kw):
    nc.tensor.matmul(out, lhsT, rhs, **kw)


def _ensure_ntff_hook():
    """Install the NTFF profile hook shim (container's antenv lacks it)."""
    if "antenv.axon_hooks" in sys.modules:
        return
    try:
        from trn_agent_boot.trn_boot import _ntff_profile_via_ctypes
    except Exception:
        return
    mod = types.ModuleType("antenv.axon_hooks")
    hook = [None]
    mod.set_axon_ntff_profile_hook = lambda h: hook.__setitem__(0, h)
    mod.get_axon_ntff_profile_hook = lambda: hook[0]
    sys.modules["antenv.axon_hooks"] = mod
    so = "/opt/axon/libaxon_pjrt.so"
    if os.path.exists(so):
        mod.set_axon_ntff_profile_hook(_ntff_profile_via_ctypes(so))


def build_nc():
    nc = bacc.Bacc("TRN2", target_bir_lowering=False, debug=False,
                   num_devices=N_CORES)

    xt_d = nc.dram_tensor("xt", [C, T], F32, kind="ExternalInput").ap()
    wqk_d = nc.dram_tensor("wqk", [C, 2 * CG], F32, kind="ExternalInput").ap()
    bqk_d = nc.dram_tensor("bqk", [2 * CG, 1], F32, kind="ExternalInput").ap()
    wv_d = nc.dram_tensor("wv", [C, HPG * 65], F32, kind="ExternalInput").ap()
    bv_d = nc.dram_tensor("bv", [1, HPG * 65], F32, kind="ExternalInput").ap()
    wp_d = nc.dram_tensor("wp", [CG, C], F32, kind="ExternalInput").ap()
    mask_d = nc.dram_tensor("mask", [4 * 128, 512], F32, kind="ExternalInput").ap()
    ones_d = nc.dram_tensor("ones", [1, 128], F32, kind="ExternalInput").ap()
    yt_d = nc.dram_tensor("yt", [C, T], F32, kind="ExternalOutput").ap()
    rec_d = nc.dram_tensor("rec_scratch", [HPG * NT, 512], F32).ap()
    den_d = nc.dram_tensor("den_scratch", [HPG * NT, 512], F32).ap()

    with tile.TileContext(nc) as tc:
        with tc.tile_pool(name="const", bufs=1) as cp:
            # ── persistent SBUF residents ──
            assert MMDT["v"] == MMDT["qk"]
            xt = [cp.tile([128, T], MMDT["qk"], tag=f"xt{k}", name=f"xt{k}") for k in range(KC)]
            wqk = [cp.tile([128, 2 * CG], MMDT["qk"], tag=f"wqk{k}", name=f"wqk{k}") for k in range(KC)]
            wv = [cp.tile([128, HPG * 65], MMDT["v"], tag=f"wv{k}", name=f"wv{k}") for k in range(KC)]
            bqk = [cp.tile([128, 1], F32, tag=f"bqk{m}", name=f"bqk{m}") for m in range(4)]
            bv = cp.tile([1, HPG * 65], MMDT["v"], tag="bv")
            wp = [cp.tile([128, C], MMDT["proj"], tag=f"wp{k}", name=f"wp{k}") for k in range(2)]
            masks = [cp.tile([128, 512], MMDT["av"], tag=f"mask{p}", name=f"mask{p}") for p in range(4)]
            ones = cp.tile([1, 128], MMDT["v"], tag="ones")
            qk = [cp.tile([128, T], MMDT["att"], tag=f"qk{m}", name=f"qk{m}") for m in range(4)]
            v_sb = [cp.tile([128, HPG * 65], MMDT["av"], tag=f"v{m}", name=f"v{m}") for m in range(T // 128)]
            outT = [cp.tile([128, T], MMDT["proj"], tag=f"outT{k}", name=f"outT{k}") for k in range(2)]

            for k in range(KC):
                nc.sync.dma_start(xt[k][:], xt_d[128 * k:128 * (k + 1), :].bitcast(MMDT['qk']))
                nc.sync.dma_start(wqk[k][:], wqk_d[128 * k:128 * (k + 1), :].bitcast(MMDT['qk']))
                nc.sync.dma_start(wv[k][:], wv_d[128 * k:128 * (k + 1), :].bitcast(MMDT['v']))
            for m in range(4):
                nc.sync.dma_start(bqk[m][:], bqk_d[128 * m:128 * (m + 1), :])
            nc.sync.dma_start(bv[:], bv_d[:].bitcast(MMDT['v']))
            for k in range(2):
                nc.sync.dma_start(wp[k][:], wp_d[128 * k:128 * (k + 1), :].bitcast(MMDT['proj']))
            for p in range(4):
                nc.sync.dma_start(masks[p][:], mask_d[128 * p:128 * (p + 1), :].bitcast(MMDT['av']))
            nc.sync.dma_start(ones[:], ones_d[:].bitcast(MMDT['v']))

            # ── stage B: qk^T [512, T] = wqk.T @ xt, bias on evacuation ──
            with tc.tile_pool(name="psB", bufs=3, space="PSUM") as psB:
                for mf in range(4):
                    for nt in range(NT):
                        ps = psB.tile([128, 512], F32)
                        for k in range(KC):
                            _mm(nc, ps[:], wqk[k][:, 128 * mf:128 * (mf + 1)],
                                xt[k][:, 512 * nt:512 * (nt + 1)], "qk",
                                start=(k == 0), stop=(k == KC - 1))
                        nc.vector.tensor_scalar_add(
                            qk[mf][:, 512 * nt:512 * (nt + 1)], ps[:],
                            bqk[mf][:])

                # ── stage C: v_aug [T, 260] = xt.T @ wv (+ ones col via K=1) ──
                for mt in range(T // 128):
                    ps = psB.tile([128, HPG * 65], F32, tag="psv")
                    for k in range(KC):
                        _mm(nc, ps[:], xt[k][:, 128 * mt:128 * (mt + 1)],
                            wv[k][:], "v", start=(k == 0), stop=False)
                    _mm(nc, ps[:], ones[:, :], bv[:], "k1", start=False,
                        stop=True)
                    nc.vector.tensor_copy(v_sb[mt][:], ps[:])

            # ── stage D: attention per (head, q-chunk) ──
            LAG = 3   # av matmul lags attT by LAG tiles to hide exp latency
            with (
                tc.tile_pool(name="psA", bufs=3, space="PSUM") as psA,
                tc.tile_pool(name="psAV", bufs=5, space="PSUM") as psAV,
                tc.tile_pool(name="expp", bufs=5) as expp,
                tc.tile_pool(name="recp", bufs=2) as recp,
                tc.tile_pool(name="bcp", bufs=2) as bcp,
            ):
                for h in range(HPG):
                    qtile, off = h // 2, 64 * (h % 2)
                    ktile = 2 + qtile
                    avs = []
                    for j in range(NT):
                        n_i = 4 * (j + 1)   # causal: k-chunks 0 .. 4j+3
                        avps = psAV.tile([65, 512], F32, tag="avps",
                                         name=f"avps{h}_{j}")
                        ets = {}
                        for i in range(n_i):
                            aps = psA.tile([128, 512], F32, tag="aps",
                                           name=f"aps{h}_{j}_{i}")
                            _mm(nc, aps[:],
                                qk[ktile][off:off + 64, 128 * i:128 * (i + 1)],
                                qk[qtile][off:off + 64, 512 * j:512 * (j + 1)],
                                "att", start=True, stop=True)
                            et = expp.tile([128, 512], MMDT["av"], tag="et",
                                           name=f"et{h}_{j}_{i}")
                            nc.scalar.activation(et[:], aps[:],
                                                 mybir.ActivationFunctionType.Exp)
                            if i >= 4 * j:  # diagonal tile: 0/1 mask post-exp
                                nc.vector.tensor_mul(et[:], et[:],
                                                     masks[i - 4 * j][:])
                            ets[i] = et
                            if i >= LAG:
                                _mm(nc, avps[:], v_sb[i - LAG][:, 65 * h:65 * h + 65],
                                    ets.pop(i - LAG)[:], "av",
                                    start=(i - LAG == 0), stop=False)
                        for i in sorted(ets):
                            _mm(nc, avps[:], v_sb[i][:, 65 * h:65 * h + 65],
                                ets.pop(i)[:], "av",
                                start=(i == 0), stop=(i == n_i - 1))
                        den1 = recp.tile([1, 512], F32, tag="den1",
                                         name=f"den1_{h}_{j}")
                        nc.vector.tensor_copy(den1[:], avps[64:65, :])
                        nc.sync.dma_start(den_d[NT * h + j:NT * h + j + 1, :],
                                          den1[:])
                        avs.append(avps)
                    den_sb = recp.tile([NT, 512], F32, tag="den_sb",
                                       name=f"den_sb{h}")
                    nc.sync.dma_start(den_sb[:], den_d[NT * h:NT * (h + 1), :])
                    rec = recp.tile([NT, 512], F32, tag="rec", name=f"rec{h}")
                    nc.vector.reciprocal(rec[:], den_sb[:])
                    nc.sync.dma_start(rec_d[NT * h:NT * (h + 1), :], rec[:])
                    for j in range(NT):
                        bc_sb = bcp.tile([64, 512], F32, tag="bc_sb",
                                         name=f"bc{h}_{j}")
                        bsrc = bass.AP(rec_d.tensor, (NT * h + j) * 512,
                                       [[0, 64], [1, 512]])
                        nc.sync.dma_start(bc_sb[:], bsrc)
                        nc.vector.tensor_mul(
                            outT[qtile][off:off + 64, 512 * j:512 * (j + 1)],
                            avs[j][0:64, :], bc_sb[:])

            # ── stage E: y^T partial [C, T] = wp.T @ outT ──
            with (
                tc.tile_pool(name="psP", bufs=3, space="PSUM") as psP,
                tc.tile_pool(name="outp", bufs=3) as outp,
            ):
                for mo in range(8):
                    for nt in range(NT):
                        ps = psP.tile([128, 512], F32)
                        for k in range(2):
                            _mm(nc, ps[:], wp[k][:, 128 * mo:128 * (mo + 1)],
                                outT[k][:, 512 * nt:512 * (nt + 1)], "proj",
                                start=(k == 0), stop=(k == 1))
                        ot = outp.tile([128, 512], F32)
                        nc.vector.tensor_copy(ot[:], ps[:])
                        nc.sync.dma_start(
                            yt_d[128 * mo:128 * (mo + 1),
                                 512 * nt:512 * (nt + 1)], ot[:])

    nc.compile()
    return nc


def _shard_inputs(x, w_qkv, b_qkv, w_proj):
    scale = 1.0 / np.sqrt(D)   # 0.125, exact power of two
    in_maps = []
    mask = np.empty((4 * 128, 512), np.float32)
    for p in range(4):
        r = np.arange(128)[:, None]
        c = np.arange(512)[None, :]
        mask[128 * p:128 * (p + 1)] = np.where(c >= r + 128 * p, 1.0, 0.0)
    for core in range(N_CORES):
        b, g = divmod(core, HPG)
        qs = slice(CG * g, CG * (g + 1))
        ks = slice(C + CG * g, C + CG * (g + 1))
        vs = slice(2 * C + CG * g, 2 * C + CG * (g + 1))
        wqk = np.concatenate([w_qkv[qs] * scale, w_qkv[ks]], axis=0).T
        bqk = np.concatenate([b_qkv[qs] * scale, b_qkv[ks]])[:, None]
        wv_base = w_qkv[vs].T          # [C, 256]
        wv = np.zeros((C, HPG * 65), np.float32)
        bv = np.zeros((1, HPG * 65), np.float32)
        for h in range(HPG):
            wv[:, 65 * h:65 * h + 64] = wv_base[:, 64 * h:64 * h + 64]
            bv[0, 65 * h:65 * h + 64] = b_qkv[vs][64 * h:64 * h + 64]
            bv[0, 65 * h + 64] = 1.0
        in_maps.append({
            "xt": np.ascontiguousarray(x[b].T, np.float32),
            "wqk": np.ascontiguousarray(wqk, np.float32),
            "bqk": np.ascontiguousarray(bqk, np.float32),
            "wv": wv,
            "bv": bv,
            "wp": np.ascontiguousarray(w_proj[:, CG * g:CG * (g + 1)].T,
                                       np.float32),
            "mask": mask,
            "ones": np.ones((1, 128), np.float32),
        })
    return in_maps


def kernel(x, w_qkv, b_qkv, w_proj, b_proj):
    x = np.asarray(x, np.float32)
    w_qkv = np.asarray(w_qkv, np.float32)
    b_qkv = np.asarray(b_qkv, np.float32)
    w_proj = np.asarray(w_proj, np.float32)
    b_proj = np.asarray(b_proj, np.float32)

    nc = build_nc()
    in_maps = _shard_inputs(x, w_qkv, b_qkv, w_proj)
    if _trace_flag[0]:
        _ensure_ntff_hook()
    res = run_bass_kernel_spmd(nc, in_maps, core_ids=list(range(N_CORES)),
                               trace=_trace_flag[0])
    _last_results[0] = res

    y = np.empty((B, T, C), np.float32)
    for b in range(B):
        acc = np.zeros((C, T), np.float32)
        for g in range(HPG):
            acc += res.results[HPG * b + g]["yt"]
        y[b] = acc.T + b_proj[None, :]
    return y


# revision 17
# speedup vs baseline: 2.7006x; 1.1152x over previous
"""Causal self-attention (B=2, T=2048, C=1024, H=16, D=64) on 8 trn2 cores.

Sharding: core c -> batch b = c // 4, head-group g = c % 4 (4 heads each).
Data-parallel over B, tensor-parallel (Megatron) over heads for the
qkv / proj linears. Each core computes its head-group's attention and a
partial output projection; the host sums the 4 partials per batch and
adds the proj bias.

Everything on-device is done in transposed [feature, token] space:
  qk^T = Wqk @ x^T                       (PE; bias added by ACT on evacuate)
  v    = x @ Wv^T (+ ones column)        (PE; K=1 matmul adds bias + ones)
  att^T[k, q] = k^T(head)^T . q(head)    (PE, K=64; causal tiles skipped)
  P = exp(att^T + additive causal mask)  (ACT; no max-subtraction needed,
                                          |logits| <~ 10 so fp32 exp is safe)
  rawout^T[d+1, q] = v_aug^T @ P         (PE accumulate over k chunks; the
                                          ones column makes row 64 = sum_k P
                                          = softmax denominator, for free)
  out^T = rawout^T[:64] * (1/denom)      (DVE recip + PE K=1 broadcast + DVE)
  y^T(partial) = Wp_g^T.T @ out^T        (PE)
"""

import os
import sys
import types

for _p in ("/opt/trn_rl_repo", "/root/.axon_site", "/root/.axon_site/_ro/trn_rl_repo"):
    if os.path.isdir(_p) and _p not in sys.path:
        sys.path.append(_p)

import numpy as np

import concourse.bacc as bacc
import concourse.bass as bass
import concourse.mybir as mybir
import concourse.tile as tile
from concourse.bass_utils import run_bass_kernel_spmd

# ── problem constants (hardcoded; spec.json not available at grade time) ──
B, T, C = 2, 2048, 1024
H, D = 16, 64
N_CORES = 8
HPG = 4                 # heads per group (per core)
CG = HPG * D            # 256 channels per head-group
NT = T // 512           # 4 token chunks of 512
KC = C // 128           # 8 contraction tiles for C
NEG = -1.0e4            # (unused) additive mask value

F32 = mybir.dt.float32
F32R = mybir.dt.float32r
# per-stage matmul operand dtype: float32 (exact, 4 cyc/row) or float32r
# (tf32-like, 1 cyc/row at N>=256). Overridable for A/B testing.
MMDT = {
    "qk": F32R, "v": F32R, "att": F32R, "av": F32R, "proj": F32R, "k1": F32,
}
_trace_flag = [False]   # test.py can flip this to capture a profile
_last_results = [None]


def _mm(nc, out, lhsT, rhs, stage, **kw):
    nc.tensor.matmul(out, lhsT, rhs, **kw)


def _ensure_ntff_hook():
    """Install the NTFF profile hook shim (container's antenv lacks it)."""
    if "antenv.axon_hooks" in sys.modules:
        return
    try:
        from trn_agent_boot.trn_boot import _ntff_profile_via_ctypes
    except Exception:
        return
    mod = types.ModuleType("antenv.axon_hooks")
    hook = [None]
    mod.set_axon_ntff_profile_hook = lambda h: hook.__setitem__(0, h)
    mod.get_axon_ntff_profile_hook = lambda: hook[0]
    sys.modules["antenv.axon_hooks"] = mod
    so = "/opt/axon/libaxon_pjrt.so"
    if os.path.exists(so):
        mod.set_axon_ntff_profile_hook(_ntff_profile_via_ctypes(so))


def build_nc():
    nc = bacc.Bacc("TRN2", target_bir_lowering=False, debug=False,
                   num_devices=N_CORES)

    xt_d = nc.dram_tensor("xt", [C, T], F32, kind="ExternalInput").ap()
    wqk_d = nc.dram_tensor("wqk", [C, 2 * CG], F32, kind="ExternalInput").ap()
    bqk_d = nc.dram_tensor("bqk", [2 * CG, 1], F32, kind="ExternalInput").ap()
    wv_d = nc.dram_tensor("wv", [C, HPG * 65], F32, kind="ExternalInput").ap()
    bv_d = nc.dram_tensor("bv", [1, HPG * 65], F32, kind="ExternalInput").ap()
    wp_d = nc.dram_tensor("wp", [CG, C], F32, kind="ExternalInput").ap()
    mask_d = nc.dram_tensor("mask", [128, 128], F32, kind="ExternalInput").ap()
    ones_d = nc.dram_tensor("ones", [1, 128], F32, kind="ExternalInput").ap()
    yt_d = nc.dram_tensor("yt", [C, T], F32, kind="ExternalOutput").ap()
    rec_d = nc.dram_tensor("rec_scratch", [HPG * NT, 512], F32).ap()
    den_d = nc.dram_tensor("den_scratch", [HPG * NT, 512], F32).ap()

    with tile.TileContext(nc) as tc:
        with tc.tile_pool(name="const", bufs=1) as cp:
            # ── persistent SBUF residents ──
            assert MMDT["v"] == MMDT["qk"]
            xt = [cp.tile([128, T], MMDT["qk"], tag=f"xt{k}", name=f"xt{k}") for k in range(KC)]
            wqk = [cp.tile([128, 2 * CG], MMDT["qk"], tag=f"wqk{k}", name=f"wqk{k}") for k in range(KC)]
            wv = [cp.tile([128, HPG * 65], MMDT["v"], tag=f"wv{k}", name=f"wv{k}") for k in range(KC)]
            bqk = [cp.tile([128, 1], F32, tag=f"bqk{m}", name=f"bqk{m}") for m in range(4)]
            bv = cp.tile([1, HPG * 65], MMDT["v"], tag="bv")
            wp = [cp.tile([128, C], MMDT["proj"], tag=f"wp{k}", name=f"wp{k}") for k in range(2)]
            tri = cp.tile([128, 128], MMDT["av"], tag="tri", name="tri")
            ones = cp.tile([1, 128], MMDT["v"], tag="ones")
            qk = [cp.tile([128, T], MMDT["att"], tag=f"qk{m}", name=f"qk{m}") for m in range(4)]
            v_sb = [cp.tile([128, HPG * 65], MMDT["av"], tag=f"v{m}", name=f"v{m}") for m in range(T // 128)]
            outT = [cp.tile([128, T], MMDT["proj"], tag=f"outT{k}", name=f"outT{k}") for k in range(2)]

            for k in range(KC):
                nc.sync.dma_start(xt[k][:], xt_d[128 * k:128 * (k + 1), :].bitcast(MMDT['qk']))
                nc.sync.dma_start(wqk[k][:], wqk_d[128 * k:128 * (k + 1), :].bitcast(MMDT['qk']))
                nc.sync.dma_start(wv[k][:], wv_d[128 * k:128 * (k + 1), :].bitcast(MMDT['v']))
            for m in range(4):
                nc.sync.dma_start(bqk[m][:], bqk_d[128 * m:128 * (m + 1), :])
            nc.sync.dma_start(bv[:], bv_d[:].bitcast(MMDT['v']))
            for k in range(2):
                nc.sync.dma_start(wp[k][:], wp_d[128 * k:128 * (k + 1), :].bitcast(MMDT['proj']))
            nc.sync.dma_start(tri[:], mask_d[:].bitcast(MMDT['av']))
            nc.sync.dma_start(ones[:], ones_d[:].bitcast(MMDT['v']))

            # ── stage B: qk^T [512, T] = wqk.T @ xt, bias on evacuation ──
            with tc.tile_pool(name="psB", bufs=3, space="PSUM") as psB:
                for mf in range(4):
                    for nt in range(NT):
                        ps = psB.tile([128, 512], F32)
                        for k in range(KC):
                            _mm(nc, ps[:], wqk[k][:, 128 * mf:128 * (mf + 1)],
                                xt[k][:, 512 * nt:512 * (nt + 1)], "qk",
                                start=(k == 0), stop=(k == KC - 1))
                        nc.vector.tensor_scalar_add(
                            qk[mf][:, 512 * nt:512 * (nt + 1)], ps[:],
                            bqk[mf][:])

                # ── stage C: v_aug [T, 260] = xt.T @ wv (+ ones col via K=1) ──
                for mt in range(T // 128):
                    ps = psB.tile([128, HPG * 65], F32, tag="psv")
                    for k in range(KC):
                        _mm(nc, ps[:], xt[k][:, 128 * mt:128 * (mt + 1)],
                            wv[k][:], "v", start=(k == 0), stop=False)
                    _mm(nc, ps[:], ones[:, :], bv[:], "k1", start=False,
                        stop=True)
                    nc.vector.tensor_copy(v_sb[mt][:], ps[:])

            # ── stage D: attention, heads paired (even head on PE rows
            # 0-63, odd head on rows 64-127 so their LDWEIGHTS/MATMULs
            # overlap across row groups) ──
            LAG = 2   # av lags attT by LAG tiles to hide the exp latency
            with (
                tc.tile_pool(name="psA", bufs=2, space="PSUM") as psA,
                tc.tile_pool(name="psAV", bufs=2, space="PSUM") as psAV,
                tc.tile_pool(name="expp", bufs=4) as expp,
                tc.tile_pool(name="recp", bufs=2) as recp,
                tc.tile_pool(name="rawp", bufs=2) as rawp,
                tc.tile_pool(name="bcp", bufs=1) as bcp,
            ):
                for hp in range(HPG // 2):
                    h0, h1 = 2 * hp, 2 * hp + 1
                    qtile, ktile = hp, 2 + hp
                    for j in range(NT):
                        u = hp * NT + j          # pair-unit index 0..7
                        n_i = 4 * (j + 1)        # causal: k-chunks 0 .. 4j+3
                        avp = [psAV.tile([65, 512], F32, tag=f"avps{s}",
                                         name=f"avps{s}_{hp}_{j}")
                               for s in range(2)]
                        ets = {}
                        for i in range(n_i):
                            p = i - 4 * j        # >=0 on diagonal tiles
                            c0 = 128 * p if p > 0 else 0   # first valid column
                            pair_et = []
                            for s, off in ((0, 0), (1, 64)):
                                aps = psA.tile([128, 512], F32, tag=f"aps{s}",
                                               name=f"aps{s}_{hp}_{j}_{i}")
                                _mm(nc, aps[:, c0:512],
                                    qk[ktile][off:off + 64, 128 * i:128 * (i + 1)],
                                    qk[qtile][off:off + 64, 512 * j + c0:512 * (j + 1)],
                                    "att", start=True, stop=True)
                                et = expp.tile([128, 512], MMDT["av"], tag=f"et{s}",
                                               name=f"et{s}_{hp}_{j}_{i}")

                                nc.scalar.activation(et[:, c0:512], aps[:, c0:512],
                                                     mybir.ActivationFunctionType.Exp)
                                if p >= 0:  # triangular block at cols [c0, c0+128)
                                    nc.vector.tensor_mul(et[:, c0:c0 + 128],
                                                         et[:, c0:c0 + 128],
                                                         tri[:])
                                pair_et.append(et)
                            ets[i] = pair_et
                            if i >= LAG:
                                ii = i - LAG
                                cc = max(0, 128 * (ii - 4 * j))
                                for s, h in ((0, h0), (1, h1)):
                                    _mm(nc, avp[s][:, cc:512],
                                        v_sb[ii][:, 65 * h:65 * h + 65],
                                        ets[ii][s][:, cc:512], "av",
                                        start=(ii == 0), stop=False)
                                del ets[ii]
                        for ii in sorted(ets):
                            cc = max(0, 128 * (ii - 4 * j))
                            for s, h in ((0, h0), (1, h1)):
                                _mm(nc, avp[s][:, cc:512],
                                    v_sb[ii][:, 65 * h:65 * h + 65],
                                    ets[ii][s][:, cc:512], "av",
                                    start=(ii == 0), stop=(ii == n_i - 1))
                            del ets[ii]
                        # evacuate rawout+denominator, free the PSUM banks
                        raw = [rawp.tile([65, 512], F32, tag=f"raw{s}",
                                         name=f"raw{s}_{hp}_{j}")
                               for s in range(2)]
                        for s in range(2):
                            nc.vector.tensor_copy(raw[s][:], avp[s][:])
                            nc.sync.dma_start(den_d[2 * u + s:2 * u + s + 1, :],
                                              raw[s][64:65, :])
                        # reciprocal of both denominator rows, remapped to
                        # [128, 8] so all DVE lanes share the work
                        den2 = recp.tile([128, 8], F32, tag="den2",
                                         name=f"den2_{hp}_{j}")
                        nc.sync.dma_start(
                            den2[:], bass.AP(den_d.tensor, 2 * u * 512,
                                             [[8, 128], [1, 8]]))
                        rec2 = recp.tile([128, 8], F32, tag="rec2",
                                         name=f"rec2_{hp}_{j}")
                        nc.vector.reciprocal(rec2[:], den2[:])
                        nc.sync.dma_start(
                            bass.AP(rec_d.tensor, 2 * u * 512, [[8, 128], [1, 8]]),
                            rec2[:])
                        for s, h in ((0, h0), (1, h1)):
                            bc_sb = bcp.tile([64, 512], F32, tag=f"bc{s}",
                                             name=f"bc{s}_{hp}_{j}")
                            nc.sync.dma_start(
                                bc_sb[:], bass.AP(rec_d.tensor, (2 * u + s) * 512,
                                                  [[0, 64], [1, 512]]))
                            off = 64 * (h % 2)
                            nc.vector.tensor_mul(
                                outT[qtile][off:off + 64, 512 * j:512 * (j + 1)],
                                raw[s][0:64, :], bc_sb[:])

            # ── stage E: y^T partial [C, T] = wp.T @ outT ──
            with (
                tc.tile_pool(name="psP", bufs=3, space="PSUM") as psP,
                tc.tile_pool(name="outp", bufs=3) as outp,
            ):
                for mo in range(8):
                    for nt in range(NT):
                        ps = psP.tile([128, 512], F32)
                        for k in range(2):
                            _mm(nc, ps[:], wp[k][:, 128 * mo:128 * (mo + 1)],
                                outT[k][:, 512 * nt:512 * (nt + 1)], "proj",
                                start=(k == 0), stop=(k == 1))
                        ot = outp.tile([128, 512], F32)
                        nc.vector.tensor_copy(ot[:], ps[:])
                        nc.sync.dma_start(
                            yt_d[128 * mo:128 * (mo + 1),
                                 512 * nt:512 * (nt + 1)], ot[:])

    nc.compile()
    return nc


def _shard_inputs(x, w_qkv, b_qkv, w_proj):
    scale = 1.0 / np.sqrt(D)   # 0.125, exact power of two
    in_maps = []
    r = np.arange(128)[:, None]
    c = np.arange(128)[None, :]
    mask = np.where(c >= r, 1.0, 0.0).astype(np.float32)
    for core in range(N_CORES):
        b, g = divmod(core, HPG)
        qs = slice(CG * g, CG * (g + 1))
        ks = slice(C + CG * g, C + CG * (g + 1))
        vs = slice(2 * C + CG * g, 2 * C + CG * (g + 1))
        wqk = np.concatenate([w_qkv[qs] * scale, w_qkv[ks]], axis=0).T
        bqk = np.concatenate([b_qkv[qs] * scale, b_qkv[ks]])[:, None]
        wv_base = w_qkv[vs].T          # [C, 256]
        wv = np.zeros((C, HPG * 65), np.float32)
        bv = np.zeros((1, HPG * 65), np.float32)
        for h in range(HPG):
            wv[:, 65 * h:65 * h + 64] = wv_base[:, 64 * h:64 * h + 64]
            bv[0, 65 * h:65 * h + 64] = b_qkv[vs][64 * h:64 * h + 64]
            bv[0, 65 * h + 64] = 1.0
        in_maps.append({
            "xt": np.ascontiguousarray(x[b].T, np.float32),
            "wqk": np.ascontiguousarray(wqk, np.float32),
            "bqk": np.ascontiguousarray(bqk, np.float32),
            "wv": wv,
            "bv": bv,
            "wp": np.ascontiguousarray(w_proj[:, CG * g:CG * (g + 1)].T,
                                       np.float32),
            "mask": mask,
            "ones": np.ones((1, 128), np.float32),
        })
    return in_maps


def kernel(x, w_qkv, b_qkv, w_proj, b_proj):
    x = np.asarray(x, np.float32)
    w_qkv = np.asarray(w_qkv, np.float32)
    b_qkv = np.asarray(b_qkv, np.float32)
    w_proj = np.asarray(w_proj, np.float32)
    b_proj = np.asarray(b_proj, np.float32)

    nc = build_nc()
    in_maps = _shard_inputs(x, w_qkv, b_qkv, w_proj)
    if _trace_flag[0]:
        _ensure_ntff_hook()
    res = run_bass_kernel_spmd(nc, in_maps, core_ids=list(range(N_CORES)),
                               trace=_trace_flag[0])
    _last_results[0] = res

    y = np.empty((B, T, C), np.float32)
    for b in range(B):
        acc = np.zeros((C, T), np.float32)
        for g in range(HPG):
            acc += res.results[HPG * b + g]["yt"]
        y[b] = acc.T + b_proj[None, :]
    return y
